# revision 11
# baseline (speedup 1.0000x reference)
"""ConvKAN Trainium2 kernel.

Decomposition (validated to 3e-7 vs reference):
  out[(b, cin, kh, kw, q), oc] =
      sum_{func, dh, dw} Wf[oc, func, dh*48+dw] * F_func(x_pad[b, cin, 12q+kh+dh, kw+dw])
  where F_0 = silu and F_{1+g}(v) = (r2^3 - 4*r1^3)/6 with
  t = |2.5 v + 3.5 - g|, r2 = max(2-t, 0), r1 = max(1-t, 0)  (uniform cubic B-spline).

Sharding: input channels cin split 8 ways (8 per core); because the reference's
"faithful" view(-1, in_dim) maps row blocks to output channels, core k produces
output channels [16k, 16k+16) for all batches.

Device layout: features packed on partitions as (func-slot, w_pad) with 64-wide
slots; contraction per matmul = (2 funcs x 64 w); accumulate over dh (12) and
5 chunk tensors into PSUM per output group (kh, kw); kw is folded into
host-prepared zero-padded weights; kh, dh, q ride the free-dim h access pattern.
"""

from contextlib import ExitStack

import numpy as np

import concourse.bass as bass
import concourse.bacc as bacc
import concourse.tile as tile
from concourse import mybir
from concourse.alu_op_type import AluOpType
from concourse.bass_utils import run_bass_kernel_spmd

AF = mybir.ActivationFunctionType
DT = mybir.dt

B, C, H, W = 16, 64, 48, 48
OUT_C = 128
NCORES = 8
CLOC = C // NCORES          # 8 input channels per core
BC = B * CLOC               # 128 (b, c) pairs per core
HP = 50                     # padded height
WSLOT = 64                  # padded width slot on partitions
FREE = HP * BC              # 6400
A3 = 4.0 ** (1.0 / 3.0)
RUN_KWARGS = {}
LAST_EXEC_NS = None
N_IDX = 3 * 12 * 5          # kw * dh * chunk lhsT tiles
# chunk -> (funcA, funcB) with func 0 = silu, 1+g = basis g; None = zero pad
CHUNK_FUNCS = [(0, None), (1, 2), (3, 4), (5, 6), (7, 8)]


def build_nc(fch: int = 4) -> bass.Bass:
    nc = bacc.Bacc(None, target_bir_lowering=False, debug=True)
    xT = nc.declare_dram_parameter("xT", [128, FREE], DT.float32, isOutput=False)
    wq = nc.declare_dram_parameter("wq", [128, N_IDX * 128], DT.float16, isOutput=False)
    bias = nc.declare_dram_parameter("bias", [128, 8], DT.float32, isOutput=False)
    out = nc.declare_dram_parameter("out", [9, 128, 512], DT.float32, isOutput=True)

    with ExitStack() as ctx:
        tc = ctx.enter_context(tile.TileContext(nc))
        wpool = ctx.enter_context(tc.tile_pool(name="w", bufs=1))
        fpool = ctx.enter_context(tc.tile_pool(name="f", bufs=2))
        psum_pool = ctx.enter_context(tc.tile_pool(name="ps", bufs=8, space="PSUM"))
        opool = ctx.enter_context(tc.tile_pool(name="o", bufs=4))

        wq_sb = wpool.tile([128, N_IDX * 128], DT.float16)
        nc.gpsimd.dma_start(wq_sb[:], wq[:])
        x_sb = wpool.tile([128, FREE], DT.float32)
        nc.gpsimd.dma_start(x_sb[:], xT[:])
        bias_sb = wpool.tile([128, 8], DT.float32)
        nc.gpsimd.dma_start(bias_sb[:], bias[:])

        ts_t = [wpool.tile([128, FREE], DT.float16, name=f"ts{k}", tag=f"ts{k}")
                for k in range(5)]

        # chunk 0: silu = x * sigmoid(x) (partitions >=64 get dup - zero weights)
        fw = FREE // fch
        for f in range(fch):
            sl = slice(f * fw, (f + 1) * fw)
            sig = fpool.tile([128, fw], DT.float32, tag="sig")
            nc.scalar.activation(sig[:], x_sb[:, sl], AF.Sigmoid)
            nc.vector.tensor_tensor(ts_t[0][:, sl], x_sb[:, sl], sig[:],
                                    op=AluOpType.mult)

        # basis chunks: TS value = 4 r1^3 - r2^3 (spline weights negated)
        for k in range(1, 5):
            for f in range(fch):
                sl = slice(f * fw, (f + 1) * fw)
                t = fpool.tile([128, fw], DT.float32, tag="t")
                nc.scalar.activation(t[:], x_sb[:, sl], AF.Abs,
                                     bias=bias_sb[:, k - 1:k], scale=2.5)
                nr2 = fpool.tile([128, fw], DT.float16, tag="nr2")  # -r2 = min(t-2, 0)
                nc.vector.tensor_scalar(nr2[:], t[:], 2.0, 0.0,
                                        op0=AluOpType.subtract, op1=AluOpType.min)
                nr1 = fpool.tile([128, fw], DT.float16, tag="nr1")  # -r1 = min(nr2+1, 0)
                nc.vector.tensor_scalar(nr1[:], nr2[:], 1.0, 0.0,
                                        op0=AluOpType.add, op1=AluOpType.min)
                s2 = fpool.tile([128, fw], DT.float16, tag="s2")    # r2^2
                nc.gpsimd.tensor_tensor(s2[:], nr2[:], nr2[:], op=AluOpType.mult)
                s1 = fpool.tile([128, fw], DT.float16, tag="s1")    # r1^2
                nc.gpsimd.tensor_tensor(s1[:], nr1[:], nr1[:], op=AluOpType.mult)
                c2n = fpool.tile([128, fw], DT.float16, tag="c2n")  # -r2^3
                nc.gpsimd.tensor_tensor(c2n[:], s2[:], nr2[:], op=AluOpType.mult)
                c1 = fpool.tile([128, fw], DT.float16, tag="c1")    # 4 r1^3
                nc.vector.scalar_tensor_tensor(c1[:], s1[:], -4.0, nr1[:],
                                               op0=AluOpType.mult, op1=AluOpType.mult)
                # 4 r1^3 - r2^3 = c1 + c2n
                nc.vector.tensor_tensor(ts_t[k][:, sl], c1[:], c2n[:],
                                        op=AluOpType.add)

        # matmuls: 9 output groups (kh, kw); accumulate 12 dh x 5 chunks.
        # Two PSUM waves; within a wave, MMs ordered chunk-major so PE can
        # start as soon as the first chunk tensor is complete.
        groups = [(kh, kw) for kh in range(3) for kw in range(3)]
        waves = [groups[:5], groups[5:]]
        ps_tiles = {}
        for wave in waves:
            for g in wave:
                ps_tiles[g] = psum_pool.tile([128, 512], DT.float32,
                                             name=f"ps_{g[0]}{g[1]}", tag="ps")
            for ch in range(5):
                for (kh, kw) in wave:
                    ps3 = ps_tiles[(kh, kw)][:].rearrange("p (q b) -> p q b", b=BC)
                    for dh in range(12):
                        h0 = kh + dh
                        idx = (kw * 12 + dh) * 5 + ch
                        lhsT = wq_sb[:, idx * 128:(idx + 1) * 128]
                        rhs = ts_t[ch][:].rearrange("p (h b) -> p h b", b=BC)[
                            :, h0:h0 + 37:12, :]
                        nc.tensor.matmul(ps3, lhsT, rhs,
                                         start=(ch == 0 and dh == 0),
                                         stop=(ch == 4 and dh == 11))
            for (kh, kw) in wave:
                ob = opool.tile([128, 512], DT.float32)
                nc.scalar.copy(ob[:], ps_tiles[(kh, kw)][:])
                nc.gpsimd.dma_start(out[kh * 3 + kw], ob[:])
    nc.compile()
    return nc


def _prep_weights(base_weight, spline_weight, spline_scaler):
    # Wf[oc, func, jj]: func 0 = silu weights, 1+g = scaled spline / 6
    wf = np.empty((OUT_C, 9, 576), dtype=np.float64)
    wf[:, 0, :] = base_weight
    wf[:, 1:, :] = np.moveaxis(
        spline_weight.astype(np.float64)
        * spline_scaler.astype(np.float64)[..., None] / -6.0, -1, 1)
    w4 = wf.reshape(OUT_C, 9, 12, 48)
    # wq[p, idx=(kw,dh,chunk), oc]
    wq = np.zeros((128, 3, 12, 5, OUT_C), dtype=np.float64)
    for kw in range(3):
        for ch, (fa, fb) in enumerate(CHUNK_FUNCS):
            for s, func in enumerate((fa, fb)):
                if func is None:
                    continue
                # partition p = s*64 + wp; weight = w4[oc, func, dh, wp-kw], 0<=wp-kw<48
                for wp in range(kw, kw + 48):
                    wq[s * 64 + wp, kw, :, ch, :] = w4[:, func, :, wp - kw].T
    wq = wq.reshape(128, 3 * 12 * 5 * 128).astype(np.float16)
    bias = np.zeros((128, 8), dtype=np.float32)
    bias[:, 4] = -A3
    for k in range(1, 5):
        ga, gb = CHUNK_FUNCS[k]
        bias[:64, k - 1] = 3.5 - (ga - 1)
        bias[64:, k - 1] = 3.5 - (gb - 1)
    return wq, bias


def _prep_x(x_slice):
    # x_slice: (B, CLOC, 48, 48) float32 -> (128, FREE) with dup halves
    xp = np.zeros((WSLOT, HP, BC), dtype=np.float32)
    xp[1:49, 1:49, :] = np.ascontiguousarray(x_slice.transpose(3, 2, 0, 1)).reshape(48, 48, BC)
    flat = xp.reshape(WSLOT, FREE)
    return np.concatenate([flat, flat], axis=0)


def kernel(x, base_weight, spline_weight, spline_scaler):
    x = np.asarray(x, dtype=np.float32)
    wq, bias = _prep_weights(np.asarray(base_weight), np.asarray(spline_weight),
                             np.asarray(spline_scaler))
    nc = build_nc()
    in_maps = []
    for k in range(NCORES):
        xs = x[:, k * CLOC:(k + 1) * CLOC]
        in_maps.append({"xT": _prep_x(xs), "wq": wq, "bias": bias})
    res = run_bass_kernel_spmd(nc, in_maps, list(range(NCORES)), **RUN_KWARGS)
    global LAST_EXEC_NS
    LAST_EXEC_NS = res.exec_time_ns
    outs = [np.asarray(r["out"]) for r in res.results]

    full = np.empty((B, 2304, OUT_C), dtype=np.float32)
    for k in range(NCORES):
        dev = outs[k].reshape(3, 3, OUT_C, 4, B, CLOC)
        rows = dev.transpose(4, 5, 0, 1, 3, 2).reshape(B, 288, OUT_C)
        full[:, 288 * k:288 * (k + 1), :] = rows
    return full.reshape(B, 128, 2304).reshape(B, 128, 48, 48)


# revision 15
# speedup vs baseline: 1.7953x; 1.7953x over previous
"""ConvKAN Trainium2 kernel.

Decomposition (validated to 3e-7 vs reference):
  out[(b, cin, kh, kw, q), oc] =
      sum_{func, dh, dw} Wf[oc, func, dh*48+dw] * F_func(x_pad[b, cin, 12q+kh+dh, kw+dw])
  where F_0 = silu and F_{1+g}(v) = (r2^3 - 4*r1^3)/6 with
  t = |2.5 v + 3.5 - g|, r2 = max(2-t, 0), r1 = max(1-t, 0)  (uniform cubic B-spline).

Sharding: input channels cin split 8 ways (8 per core); because the reference's
"faithful" view(-1, in_dim) maps row blocks to output channels, core k produces
output channels [16k, 16k+16) for all batches.

Device layout: features packed on partitions as (func-slot, w_pad) with 64-wide
slots; contraction per matmul = (2 funcs x 64 w); accumulate over dh (12) and
5 chunk tensors into PSUM per output group (kh, kw); kw is folded into
host-prepared zero-padded weights; kh, dh, q ride the free-dim h access pattern.
"""

from contextlib import ExitStack

import numpy as np

import concourse.bass as bass
import concourse.bacc as bacc
import concourse.tile as tile
from concourse import mybir
from concourse.alu_op_type import AluOpType
from concourse.bass_utils import run_bass_kernel_spmd

AF = mybir.ActivationFunctionType
DT = mybir.dt

B, C, H, W = 16, 64, 48, 48
OUT_C = 128
NCORES = 8
CLOC = C // NCORES          # 8 input channels per core
BC = B * CLOC               # 128 (b, c) pairs per core
HP = 50                     # padded height
WSLOT = 64                  # padded width slot on partitions
FREE = HP * BC              # 6400
A3 = 4.0 ** (1.0 / 3.0)
RUN_KWARGS = {}
LAST_EXEC_NS = None
N_IDX = 3 * 12 * 5          # kw * dh * chunk lhsT tiles
# chunk -> (funcA, funcB) with func 0 = silu, 1+g = basis g; None = zero pad
CHUNK_FUNCS = [(0, None), (1, 2), (3, 4), (5, 6), (7, 8)]


def build_nc(fch: int = 8) -> bass.Bass:
    nc = bacc.Bacc(None, target_bir_lowering=False, debug=True)
    xT = nc.declare_dram_parameter("xT", [128, FREE], DT.float32, isOutput=False)
    wq = nc.declare_dram_parameter("wq", [128, N_IDX * 128], DT.float16, isOutput=False)
    bias = nc.declare_dram_parameter("bias", [128, 8], DT.float32, isOutput=False)
    out = nc.declare_dram_parameter("out", [9, 128, 512], DT.float32, isOutput=True)

    with ExitStack() as ctx:
        tc = ctx.enter_context(tile.TileContext(nc))
        wpool = ctx.enter_context(tc.tile_pool(name="w", bufs=1))
        fpool = ctx.enter_context(tc.tile_pool(name="f", bufs=2))
        psum_pool = ctx.enter_context(tc.tile_pool(name="ps", bufs=8, space="PSUM"))
        opool = ctx.enter_context(tc.tile_pool(name="o", bufs=4))

        wq_sb = wpool.tile([128, N_IDX * 128], DT.float16)
        nc.sync.dma_start(wq_sb[:], wq[:])
        x_sb = wpool.tile([128, FREE], DT.float32)
        nc.sync.dma_start(x_sb[:], xT[:])
        bias_sb = wpool.tile([128, 8], DT.float32)
        nc.gpsimd.dma_start(bias_sb[:], bias[:])

        ts_t = [wpool.tile([128, FREE], DT.float16, name=f"ts{k}", tag=f"ts{k}")
                for k in range(5)]

        # chunk 0: silu = x * sigmoid(x) (partitions >=64 get dup - zero weights)
        fw = FREE // fch
        for f in range(fch):
            sl = slice(f * fw, (f + 1) * fw)
            sig = fpool.tile([128, fw], DT.float32, tag="sig")
            nc.scalar.activation(sig[:], x_sb[:, sl], AF.Sigmoid)
            nc.vector.tensor_tensor(ts_t[0][:, sl], x_sb[:, sl], sig[:],
                                    op=AluOpType.mult)

        # basis chunks: TS value = 4 r1^3 - r2^3 (spline weights negated)
        for k in range(1, 5):
            for f in range(fch):
                sl = slice(f * fw, (f + 1) * fw)
                t = fpool.tile([128, fw], DT.float16, tag="t")
                nc.scalar.activation(t[:], x_sb[:, sl], AF.Abs,
                                     bias=bias_sb[:, k - 1:k], scale=2.5)
                nr2 = fpool.tile([128, fw], DT.float16, tag="nr2")  # -r2 = min(t-2, 0)
                nc.vector.tensor_scalar(nr2[:], t[:], 2.0, 0.0,
                                        op0=AluOpType.subtract, op1=AluOpType.min)
                nr1 = fpool.tile([128, fw], DT.float16, tag="nr1")  # -r1 = min(t-1, 0)
                nc.vector.tensor_scalar(nr1[:], t[:], 1.0, 0.0,
                                        op0=AluOpType.subtract, op1=AluOpType.min)
                s2 = fpool.tile([128, fw], DT.float16, tag="s2")    # r2^2
                nc.vector.tensor_tensor(s2[:], nr2[:], nr2[:], op=AluOpType.mult)
                s1f = fpool.tile([128, fw], DT.float16, tag="s1f")  # 4 r1^2
                nc.scalar.activation(s1f[:], nr1[:], AF.Square, scale=2.0)
                c2n = fpool.tile([128, fw], DT.float16, tag="c2n")  # -r2^3
                nc.vector.tensor_tensor(c2n[:], s2[:], nr2[:], op=AluOpType.mult)
                cn1 = fpool.tile([128, fw], DT.float16, tag="cn1")  # -4 r1^3
                nc.vector.tensor_tensor(cn1[:], s1f[:], nr1[:], op=AluOpType.mult)
                # 4 r1^3 - r2^3 = c2n - cn1
                nc.vector.tensor_tensor(ts_t[k][:, sl], c2n[:], cn1[:],
                                        op=AluOpType.subtract)

        # matmuls: 9 output groups (kh, kw); accumulate 12 dh x 5 chunks.
        # Two PSUM waves; within a wave, MMs ordered chunk-major so PE can
        # start as soon as the first chunk tensor is complete.
        groups = [(kh, kw) for kh in range(3) for kw in range(3)]
        waves = [groups[:4], groups[4:8], groups[8:]]
        ps_tiles = {}

        def emit_wave(wave):
            for ch in range(5):
                for (kh, kw) in wave:
                    ps3 = ps_tiles[(kh, kw)][:].rearrange("p (q b) -> p q b", b=BC)
                    for dh in range(12):
                        h0 = kh + dh
                        idx = (kw * 12 + dh) * 5 + ch
                        lhsT = wq_sb[:, idx * 128:(idx + 1) * 128]
                        rhs = ts_t[ch][:].rearrange("p (h b) -> p h b", b=BC)[
                            :, h0:h0 + 37:12, :]
                        nc.tensor.matmul(ps3, lhsT, rhs,
                                         start=(ch == 0 and dh == 0),
                                         stop=(ch == 4 and dh == 11))

        def drain_wave(wave):
            for (kh, kw) in wave:
                ob = opool.tile([128, 512], DT.float32)
                nc.scalar.copy(ob[:], ps_tiles[(kh, kw)][:])
                nc.gpsimd.dma_start(out[kh * 3 + kw], ob[:])

        for g in waves[0] + waves[1]:
            ps_tiles[g] = psum_pool.tile([128, 512], DT.float32,
                                         name=f"ps_{g[0]}{g[1]}", tag="ps")
        # interleave the two leading waves chunk-by-chunk
        for ch in range(5):
            for wave in (waves[0], waves[1]):
                for (kh, kw) in wave:
                    ps3 = ps_tiles[(kh, kw)][:].rearrange("p (q b) -> p q b", b=BC)
                    for dh in range(12):
                        h0 = kh + dh
                        idx = (kw * 12 + dh) * 5 + ch
                        lhsT = wq_sb[:, idx * 128:(idx + 1) * 128]
                        rhs = ts_t[ch][:].rearrange("p (h b) -> p h b", b=BC)[
                            :, h0:h0 + 37:12, :]
                        nc.tensor.matmul(ps3, lhsT, rhs,
                                         start=(ch == 0 and dh == 0),
                                         stop=(ch == 4 and dh == 11))
        drain_wave(waves[0])
        drain_wave(waves[1])
        for g in waves[2]:
            ps_tiles[g] = psum_pool.tile([128, 512], DT.float32,
                                         name=f"ps_{g[0]}{g[1]}", tag="ps")
        emit_wave(waves[2])
        drain_wave(waves[2])
    nc.compile()
    return nc


def _prep_weights(base_weight, spline_weight, spline_scaler):
    # Wf[oc, func, jj]: func 0 = silu weights, 1+g = scaled spline / 6
    wf = np.empty((OUT_C, 9, 576), dtype=np.float64)
    wf[:, 0, :] = base_weight
    wf[:, 1:, :] = np.moveaxis(
        spline_weight.astype(np.float64)
        * spline_scaler.astype(np.float64)[..., None] / -6.0, -1, 1)
    w4 = wf.reshape(OUT_C, 9, 12, 48)
    # wq[p, idx=(kw,dh,chunk), oc]
    wq = np.zeros((128, 3, 12, 5, OUT_C), dtype=np.float64)
    for kw in range(3):
        for ch, (fa, fb) in enumerate(CHUNK_FUNCS):
            for s, func in enumerate((fa, fb)):
                if func is None:
                    continue
                # partition p = s*64 + wp; weight = w4[oc, func, dh, wp-kw], 0<=wp-kw<48
                for wp in range(kw, kw + 48):
                    wq[s * 64 + wp, kw, :, ch, :] = w4[:, func, :, wp - kw].T
    wq = wq.reshape(128, 3 * 12 * 5 * 128).astype(np.float16)
    bias = np.zeros((128, 8), dtype=np.float32)
    bias[:, 4] = -A3
    for k in range(1, 5):
        ga, gb = CHUNK_FUNCS[k]
        bias[:64, k - 1] = 3.5 - (ga - 1)
        bias[64:, k - 1] = 3.5 - (gb - 1)
    return wq, bias


def _prep_x(x_slice):
    # x_slice: (B, CLOC, 48, 48) float32 -> (128, FREE) with dup halves
    xp = np.zeros((WSLOT, HP, BC), dtype=np.float32)
    xp[1:49, 1:49, :] = np.ascontiguousarray(x_slice.transpose(3, 2, 0, 1)).reshape(48, 48, BC)
    flat = xp.reshape(WSLOT, FREE)
    return np.concatenate([flat, flat], axis=0)


def kernel(x, base_weight, spline_weight, spline_scaler):
    x = np.asarray(x, dtype=np.float32)
    wq, bias = _prep_weights(np.asarray(base_weight), np.asarray(spline_weight),
                             np.asarray(spline_scaler))
    nc = build_nc()
    in_maps = []
    for k in range(NCORES):
        xs = x[:, k * CLOC:(k + 1) * CLOC]
        in_maps.append({"xT": _prep_x(xs), "wq": wq, "bias": bias})
    res = run_bass_kernel_spmd(nc, in_maps, list(range(NCORES)), **RUN_KWARGS)
    global LAST_EXEC_NS
    LAST_EXEC_NS = res.exec_time_ns
    outs = [np.asarray(r["out"]) for r in res.results]

    full = np.empty((B, 2304, OUT_C), dtype=np.float32)
    for k in range(NCORES):
        dev = outs[k].reshape(3, 3, OUT_C, 4, B, CLOC)
        rows = dev.transpose(4, 5, 0, 1, 3, 2).reshape(B, 288, OUT_C)
        full[:, 288 * k:288 * (k + 1), :] = rows
    return full.reshape(B, 128, 2304).reshape(B, 128, 48, 48)
